# revision 2
# baseline (speedup 1.0000x reference)
"""CLIP causal attention (B=8, T=1024, E=768, H=12) on 8 TRN2 NeuronCores.

Strategy: pure data-parallel over batch — core b handles x[b] end to end,
no collectives. All compute in transposed space (embed on partitions):

  X' = x_b^T                       [768, 1024]  (host pre-transposed, bf16)
  Q' = Wq^T @ X' (+bq)             [768, 1024]  lhsT = Wq blocks (host-packed)
  K' = Wk^T @ X' (+bk)             [768, 1024]
  V  = X'^T @ Wv_aug (+bv_aug)     [1024, 780]  per head 65 cols: 64 dims +
                                   a ones column (Wv col = 0, bias = 1) that
                                   the PV matmul turns into the softmax denom
  per head h (KQ orientation, j on partitions, i free):
     S'[j,i] = K'_h[:,jblk]^T @ Q'_h          (K=64)
     P' = exp(S' * 1/8)  (no max-subtraction: |S'/8| <= ~7, exact-safe)
     causal: skip fully-masked blocks, restrict to valid cols, tri-mask diag
     O_aug[d,i] = sum_j Vaug_h[j,:65]^T @ P'  (row 64 = softmax denominator)
     O'_h = O_aug[0:64] * broadcast(1/denom)
  out^T = Wo^T @ O' (+bo)          [768, 1024]  lhsT = Wo blocks -> transposed
                                   output; host transposes back. Bias is
                                   per-partition -> native tensor_scalar_add.

Input DMAs are host-packed into SBUF-layout contiguous tensors and issued
across three hardware queues (sync: X' + Wo + output, scalar: consts + Wq/Wk,
gpsimd: bv + Wv) so descriptor issue doesn't serialize and V-projection
compute starts ~2us in. Output is bf16 (host upcasts) to halve the tail DMA.
All matmul operands bf16 (fp32 PSUM accumulation).
"""

import numpy as np
import ml_dtypes

E = 768
T = 1024
B = 8
H = 12
DH = 64
NT = E // 128          # 6 partition-tiles of the embed dim
NJ = T // 128          # 8 partition-tiles of the token dim
SCALE = DH ** -0.5     # folded into the exp() activation's scale operand
VW = H * 65            # V_aug row width: 12 heads x (64 dims + ones col)

_CACHE = {}


def _build():
    import concourse.bass as bass
    import concourse.tile as tile
    from concourse import bacc, mybir

    f32 = mybir.dt.float32
    bf16 = mybir.dt.bfloat16
    Exp = mybir.ActivationFunctionType.Exp

    nc = bacc.Bacc(
        "TRN2",
        target_bir_lowering=False,
        debug=False,
        enable_asserts=False,
        num_devices=B,
    )

    xt = nc.dram_tensor("xt", [128, NT * T], bf16, kind="ExternalInput").ap()
    wq = nc.dram_tensor("wq", [128, NT * E], bf16, kind="ExternalInput").ap()
    wk = nc.dram_tensor("wk", [128, NT * E], bf16, kind="ExternalInput").ap()
    wv = nc.dram_tensor("wv", [128, NT * VW], bf16, kind="ExternalInput").ap()
    wo = nc.dram_tensor("wo", [128, NT * E], bf16, kind="ExternalInput").ap()
    bqt = nc.dram_tensor("bqt", [128, NT], f32, kind="ExternalInput").ap()
    bkt = nc.dram_tensor("bkt", [128, NT], f32, kind="ExternalInput").ap()
    bot = nc.dram_tensor("bot", [128, NT], f32, kind="ExternalInput").ap()
    bvb = nc.dram_tensor("bvb", [128, VW], bf16, kind="ExternalInput").ap()
    tri = nc.dram_tensor("tri", [128, 128], bf16, kind="ExternalInput").ap()
    out_t = nc.dram_tensor("out_t", [E, T], bf16, kind="ExternalOutput").ap()

    with tile.TileContext(nc) as tc:
        with (
            tc.tile_pool(name="const", bufs=1) as cpool,
            tc.tile_pool(name="psb", bufs=8) as ppool,
            tc.tile_pool(name="rsb", bufs=4) as rpool,
            tc.tile_pool(name="rbsb", bufs=4) as rbpool,
            tc.tile_pool(name="fin", bufs=3) as fpool,
            tc.tile_pool(name="pp", bufs=2, space="PSUM") as pp,
            tc.tile_pool(name="sp", bufs=2, space="PSUM") as sp,
            tc.tile_pool(name="op", bufs=2, space="PSUM") as op,
        ):
            XT = cpool.tile([128, NT * T], bf16)     # (kt, i)
            WQ = cpool.tile([128, NT * E], bf16)     # (nt, kt, c): lhsT blocks
            WK = cpool.tile([128, NT * E], bf16)
            WV = cpool.tile([128, NT * VW], bf16)    # (kt, h*65+c); col 64 of
                                                     # each head block = 0
            WO = cpool.tile([128, NT * E], bf16)     # (nt, et, c): lhsT blocks
            QS = cpool.tile([128, NT * T], bf16)     # Q' (nt, i)
            KS = cpool.tile([128, NT * T], bf16)
            VS = cpool.tile([128, NJ * VW], bf16)    # (jt, h*65+d); col 64 of
                                                     # each head block = denom ones
            OS = cpool.tile([128, NT * T], bf16)     # O' (et, i)
            BQ = cpool.tile([128, NT], f32)
            BK = cpool.tile([128, NT], f32)
            BO = cpool.tile([128, NT], f32)
            BVB = cpool.tile([128, VW], bf16)        # bv_aug pre-broadcast (host)
            TRI = cpool.tile([128, 128], bf16)

            # ---- input DMAs across three queues, priority order. sync: X'
            # per-k-tile (V projection starts as chunks land) then Wo (needed
            # last). scalar: small consts then Wq/Wk. gpsimd: bv_aug + Wv.
            nc.scalar.dma_start(BQ[:], bqt)
            nc.scalar.dma_start(BK[:], bkt)
            nc.scalar.dma_start(BO[:], bot)
            nc.scalar.dma_start(TRI[:], tri)
            nc.gpsimd.dma_start(BVB[:], bvb)
            for kt in range(NT):
                nc.sync.dma_start(XT[:, kt * T : (kt + 1) * T], xt[:, kt * T : (kt + 1) * T])
                nc.gpsimd.dma_start(WV[:, kt * VW : (kt + 1) * VW], wv[:, kt * VW : (kt + 1) * VW])
            nc.scalar.dma_start(WQ[:], wq)
            nc.scalar.dma_start(WK[:], wk)
            nc.sync.dma_start(WO[:], wo)

            # ---- PE warmup: dummy matmuls with no DMA dependency so the
            # HAM activity monitor lifts the 1.2GHz cold gate before real
            # work arrives (DUM memset first: it gates the dummies) ----
            DUMW = cpool.tile([128, 128], bf16)
            DUMR = cpool.tile([128, 512], bf16)
            nc.vector.memset(DUMW[:], 1.0)
            nc.vector.memset(DUMR[:], 1.0)

            def dummy(n=512):
                # full-array junk matmul: the HAM activity monitor only lifts
                # the 1.2GHz cold gate for real array occupancy.
                d_ps = pp.tile([128, 512], f32, tag="proj")
                nc.tensor.matmul(
                    d_ps[:, :n], lhsT=DUMW[:], rhs=DUMR[:, :n], start=True, stop=True
                )

            for _ in range(16):
                dummy()

            # ---- V projection: lhsT = X'[kt, jblk] -> V_aug[j, h*65+c].
            # The ones column comes out of the projection itself (Wv col 0,
            # bias 1), so evictions are plain contiguous adds.
            for jt in range(NJ):
                for half in range(2):
                    e0 = half * 390
                    ps = pp.tile([128, 512], f32, tag="proj")
                    for kt in range(NT):
                        nc.tensor.matmul(
                            ps[:, :390],
                            lhsT=XT[:, kt * T + jt * 128 : kt * T + jt * 128 + 128],
                            rhs=WV[:, kt * VW + e0 : kt * VW + e0 + 390],
                            start=(kt == 0),
                            stop=(kt == NT - 1),
                        )
                    nc.vector.tensor_add(
                        VS[:, jt * VW + e0 : jt * VW + e0 + 390],
                        ps[:, :390],
                        BVB[:, e0 : e0 + 390],
                    )

            # ---- Q'/K' projection for one 128-row block nt (2 heads) ----
            def qk_proj(nt):
                for W, Bb, DST in ((WQ, BQ, QS), (WK, BK, KS)):
                    for ic in range(2):
                        ps = pp.tile([128, 512], f32, tag="proj")
                        for kt in range(NT):
                            nc.tensor.matmul(
                                ps[:],
                                lhsT=W[:, nt * E + kt * 128 : nt * E + kt * 128 + 128],
                                rhs=XT[:, kt * T + ic * 512 : kt * T + ic * 512 + 512],
                                start=(kt == 0),
                                stop=(kt == NT - 1),
                            )
                        nc.vector.tensor_scalar_add(
                            DST[:, nt * T + ic * 512 : nt * T + ic * 512 + 512],
                            ps[:],
                            Bb[:, nt : nt + 1],
                        )

            def normalize(o_ps, h, ic):
                # softmax denominators live in row 64 (the V_aug ones column).
                # Full-precision reciprocal costs 3.35us on DVE; the ~18-bit
                # approx is plenty, but its BITWISE_NOT seed needs an SBUF
                # operand on hardware, so stage the PSUM row out first.
                nt, po = h // 2, (h % 2) * 64
                dn = rpool.tile([1, 512], f32, tag="denom")
                nc.vector.tensor_copy(dn[:], o_ps[64:65, :])
                r = rpool.tile([1, 512], f32, tag="recip")
                nc.vector.reciprocal_approx_fast(r[:], dn[:])
                rb = rbpool.tile([64, 512], f32, tag="recipb")
                nc.gpsimd.partition_broadcast(rb[:], r[:])
                nc.vector.tensor_mul(
                    OS[po : po + 64, nt * T + ic * 512 : nt * T + ic * 512 + 512],
                    o_ps[0:64, :],
                    rb[:],
                )

            # ---- attention for one head, KQ orientation. Per j-tile one
            # 2-bank [128,1024] scores tile spanning both i-chunks: one exp
            # per j-tile over the whole valid column range, and the K/V
            # weight loads shared by the paired matmuls. o_ps0 (i<512)
            # finishes at jt=3; its normalize overlaps the jt>=4 tail. ----
            def head(h):
                nt, po = h // 2, (h % 2) * 64
                o_ps0 = op.tile([128, 512], f32, tag="oaug")
                o_ps1 = op.tile([128, 512], f32, tag="oaug")
                for jt in range(NJ):
                    d0 = jt * 128  # first valid (global) column of this j-tile
                    s2 = sp.tile([128, 1024], f32, tag="scores")
                    p2 = ppool.tile([128, 1024], bf16, tag="probs")
                    lhsK = KS[po : po + 64, nt * T + jt * 128 : nt * T + jt * 128 + 128]
                    if jt < 4:  # contributes to both i-chunks
                        nc.tensor.matmul(
                            s2[:, d0:512],
                            lhsT=lhsK,
                            rhs=QS[po : po + 64, nt * T + d0 : nt * T + 512],
                            start=True,
                            stop=True,
                        )
                        nc.tensor.matmul(
                            s2[:, 512:1024],
                            lhsT=lhsK,
                            rhs=QS[po : po + 64, nt * T + 512 : nt * T + 1024],
                            start=True,
                            stop=True,
                        )
                    else:
                        nc.tensor.matmul(
                            s2[:, d0:1024],
                            lhsT=lhsK,
                            rhs=QS[po : po + 64, nt * T + d0 : nt * T + 1024],
                            start=True,
                            stop=True,
                        )
                    if h >= 8:
                        dummy()
                    nc.scalar.activation(p2[:, d0:1024], s2[:, d0:1024], Exp, scale=SCALE)
                    # causal triangle on the diag block. NOT gpsimd
                    # affine_select: mixing custom-op types on GpSimd forces
                    # MODIFY_POOL_CONFIG switches that stall partition_broadcast
                    nc.vector.tensor_mul(
                        p2[:, d0 : d0 + 128], p2[:, d0 : d0 + 128], TRI[:]
                    )
                    lhsV = VS[:, jt * VW + h * 65 : jt * VW + h * 65 + 65]
                    if jt < 4:
                        nc.tensor.matmul(
                            o_ps0[0:65, d0:512],
                            lhsT=lhsV,
                            rhs=p2[:, d0:512],
                            start=(jt == 0),
                            stop=(jt == 3),
                            skip_group_check=True,
                        )
                    nc.tensor.matmul(
                        o_ps1[0:65, max(0, d0 - 512) : 512],
                        lhsT=lhsV,
                        rhs=p2[:, max(512, d0) : 1024],
                        start=(jt == 0),
                        stop=(jt == NJ - 1),
                        skip_group_check=True,
                    )
                    if jt == 3:
                        normalize(o_ps0, h, 0)
                normalize(o_ps1, h, 1)

            # Interleave: each nt's Q/K projection feeds its two heads; the
            # next nt's projection matmuls keep PE busy while ScalarE runs
            # this pair's exps.
            for nt in range(NT):
                qk_proj(nt)
                head(2 * nt)
                head(2 * nt + 1)

            # ---- output projection, transposed: out^T = Wo^T @ O' (+bo).
            # lhsT = Wo[et-block, nt-block] (host-packed), rhs = O' i-chunks.
            # Bias is per-partition -> fused into the eviction tensor_scalar.
            # bf16 out_t, host transposes/upcasts.
            for nt in range(NT):
                fin = fpool.tile([128, T], bf16, tag="fin")
                for ic in range(2):
                    f_ps = pp.tile([128, 512], f32, tag="proj")
                    for et in range(NT):
                        nc.tensor.matmul(
                            f_ps[:],
                            lhsT=WO[:, nt * E + et * 128 : nt * E + et * 128 + 128],
                            rhs=OS[:, et * T + ic * 512 : et * T + ic * 512 + 512],
                            start=(et == 0),
                            stop=(et == NT - 1),
                        )
                    nc.vector.tensor_scalar_add(
                        fin[:, ic * 512 : (ic + 1) * 512], f_ps[:], BO[:, nt : nt + 1]
                    )
                nc.sync.dma_start(out_t[nt * 128 : (nt + 1) * 128, :], fin[:])

    nc.compile()
    return nc


def _get_nc():
    if "nc" not in _CACHE:
        _CACHE["nc"] = _build()
    return _CACHE["nc"]


def _pack_w(w):
    # [768, 768] -> [128, nt*768 + kt*128 + c] = w[kt*128+p, nt*128+c]
    return np.ascontiguousarray(
        w.reshape(NT, 128, NT, 128).transpose(1, 2, 0, 3).reshape(128, NT * E)
    )


def _make_in_maps(inputs):
    bf = ml_dtypes.bfloat16
    x = np.asarray(inputs["x"], np.float32)
    wv4 = np.asarray(inputs["Wv"], np.float32).reshape(E, H, DH)
    wv_aug = np.zeros((E, H, 65), np.float32)
    wv_aug[:, :, :DH] = wv4
    bv_aug = np.zeros((H, 65), np.float32)
    bv_aug[:, :DH] = np.asarray(inputs["bv"], np.float32).reshape(H, DH)
    bv_aug[:, DH] = 1.0
    shared = {
        "wq": _pack_w(np.asarray(inputs["Wq"], np.float32)).astype(bf),
        "wk": _pack_w(np.asarray(inputs["Wk"], np.float32)).astype(bf),
        "wo": _pack_w(np.asarray(inputs["Wo"], np.float32)).astype(bf),
        "wv": np.ascontiguousarray(
            wv_aug.reshape(NT, 128, VW).transpose(1, 0, 2).reshape(128, NT * VW)
        ).astype(bf),
        "bvb": np.ascontiguousarray(
            np.broadcast_to(bv_aug.reshape(1, VW), (128, VW))
        ).astype(bf),
        "bqt": np.ascontiguousarray(
            np.asarray(inputs["bq"], np.float32).reshape(NT, 128).T
        ),
        "bkt": np.ascontiguousarray(
            np.asarray(inputs["bk"], np.float32).reshape(NT, 128).T
        ),
        "bot": np.ascontiguousarray(
            np.asarray(inputs["bo"], np.float32).reshape(NT, 128).T
        ),
        "tri": np.triu(np.ones((128, 128), np.float32)).astype(bf),
    }
    xs = []
    for b in range(B):
        xp = np.ascontiguousarray(
            x[b].T.reshape(NT, 128, T).transpose(1, 0, 2).reshape(128, NT * T)
        ).astype(bf)
        xs.append(dict(shared, xt=xp))
    return xs


def _run(inputs, trace=False):
    from concourse import bass_utils

    nc = _get_nc()
    res = bass_utils.run_bass_kernel_spmd(
        nc, _make_in_maps(inputs), core_ids=list(range(B)), trace=trace
    )
    out = np.stack(
        [np.asarray(res.results[c]["out_t"]).astype(np.float32).T for c in range(B)]
    )
    return out, res


def kernel(**inputs) -> np.ndarray:
    out, _ = _run(inputs, trace=False)
    return out


# revision 10
# speedup vs baseline: 1.0706x; 1.0706x over previous
"""CLIP causal attention (B=8, T=1024, E=768, H=12) on 8 TRN2 NeuronCores.

Strategy: pure data-parallel over batch — core b handles x[b] end to end,
no collectives. All compute in transposed space (embed on partitions):

  X' = x_b^T                       [768, 1024]  (host pre-transposed, bf16)
  Q' = Wq^T @ X' (+bq)             [768, 1024]  lhsT = Wq blocks (host-packed)
  K' = Wk^T @ X' (+bk)             [768, 1024]
  V  = X'^T @ Wv_aug (+bv_aug)     [1024, 780]  per head 65 cols: 64 dims +
                                   a ones column (Wv col = 0, bias = 1) that
                                   the PV matmul turns into the softmax denom
  per head PAIR (2nt, 2nt+1) (KQ orientation, j on partitions, i free):
     the two heads' score matmuls have K=64 (head dim) and live on disjoint
     SBUF partition halves (even: 0-63, odd: 64-127), so bass auto-derives
     tile_position (0,0)/(64,0): emitted back-to-back they run CONCURRENTLY
     on the two 64-row halves of the PE array (2x on the QK^T stage).
     Per 512-wide i-chunk: phase A fills one [128,1024] PSUM tile per j-tile
     (cols 0:512 even head, 512:1024 odd head -> different banks), one
     merged exp() + one merged tri-mask per j-tile covers both heads;
     phase B runs the PV matmuls (K=128, full array) off the SBUF P tiles.
     P' = exp(S' * 1/8)  (no max-subtraction: |S'/8| <= ~7, exact-safe)
     causal: skip fully-masked blocks, restrict to valid cols, tri-mask diag
     O_aug[d,i] = sum_j Vaug_h[j,:65]^T @ P'  (row 64 = softmax denominator)
     O'_h = O_aug[0:64] * broadcast(1/denom)
  out^T = Wo^T @ O' (+bo)          [768, 1024]  lhsT = Wo blocks -> transposed
                                   output; host transposes back. Bias is
                                   per-partition -> native tensor_scalar_add.

Input DMAs are host-packed into SBUF-layout contiguous tensors and issued
across three hardware queues (sync: X' + Wo + output, scalar: consts + Wq/Wk,
gpsimd: bv + Wv) so descriptor issue doesn't serialize and V-projection
compute starts ~2us in. Output is bf16 (host upcasts) to halve the tail DMA.
All matmul operands bf16 (fp32 PSUM accumulation).
"""

import numpy as np
import ml_dtypes

E = 768
T = 1024
B = 8
H = 12
DH = 64
NT = E // 128          # 6 partition-tiles of the embed dim
NJ = T // 128          # 8 partition-tiles of the token dim
SCALE = DH ** -0.5     # folded into the exp() activation's scale operand
VW = H * 65            # V_aug row width: 12 heads x (64 dims + ones col)

_CACHE = {}


def _build():
    import concourse.bass as bass
    import concourse.tile as tile
    from concourse import bacc, mybir

    f32 = mybir.dt.float32
    bf16 = mybir.dt.bfloat16
    Exp = mybir.ActivationFunctionType.Exp

    nc = bacc.Bacc(
        "TRN2",
        target_bir_lowering=False,
        debug=False,
        enable_asserts=False,
        num_devices=B,
    )

    xt = nc.dram_tensor("xt", [128, NT * T], bf16, kind="ExternalInput").ap()
    wq = nc.dram_tensor("wq", [128, NT * E], bf16, kind="ExternalInput").ap()
    wk = nc.dram_tensor("wk", [128, NT * E], bf16, kind="ExternalInput").ap()
    wv = nc.dram_tensor("wv", [128, NT * VW], bf16, kind="ExternalInput").ap()
    wo = nc.dram_tensor("wo", [128, NT * E], bf16, kind="ExternalInput").ap()
    bqt = nc.dram_tensor("bqt", [128, NT], f32, kind="ExternalInput").ap()
    bkt = nc.dram_tensor("bkt", [128, NT], f32, kind="ExternalInput").ap()
    bot = nc.dram_tensor("bot", [128, NT], f32, kind="ExternalInput").ap()
    bvb = nc.dram_tensor("bvb", [128, VW], bf16, kind="ExternalInput").ap()
    tri = nc.dram_tensor("tri", [128, 256], bf16, kind="ExternalInput").ap()
    out_t = nc.dram_tensor("out_t", [E, T], bf16, kind="ExternalOutput").ap()

    with tile.TileContext(nc) as tc:
        with (
            tc.tile_pool(name="const", bufs=1) as cpool,
            tc.tile_pool(name="psb", bufs=12) as ppool,
            tc.tile_pool(name="rsb", bufs=4) as rpool,
            tc.tile_pool(name="rbsb", bufs=4) as rbpool,
            tc.tile_pool(name="fin", bufs=3) as fpool,
            tc.tile_pool(name="pp", bufs=2, space="PSUM") as pp,
            tc.tile_pool(name="sp", bufs=2, space="PSUM") as sp,
            tc.tile_pool(name="op", bufs=2, space="PSUM") as op,
        ):
            XT = cpool.tile([128, NT * T], bf16)     # (kt, i)
            WQ = cpool.tile([128, NT * E], bf16)     # (nt, kt, c): lhsT blocks
            WK = cpool.tile([128, NT * E], bf16)
            WV = cpool.tile([128, NT * VW], bf16)    # (kt, h*65+c); col 64 of
                                                     # each head block = 0
            WO = cpool.tile([128, NT * E], bf16)     # (nt, et, c): lhsT blocks
            QS = cpool.tile([128, NT * T], bf16)     # Q' (nt, i)
            KS = cpool.tile([128, NT * T], bf16)
            VS = cpool.tile([128, NJ * VW], bf16)    # (jt, h*65+d); col 64 of
                                                     # each head block = denom ones
            OS = cpool.tile([128, NT * T], bf16)     # O' (et, i)
            BQ = cpool.tile([128, NT], f32)
            BK = cpool.tile([128, NT], f32)
            BO = cpool.tile([128, NT], f32)
            BVB = cpool.tile([128, VW], bf16)        # bv_aug pre-broadcast (host)
            TRI2 = cpool.tile([128, 256], bf16)      # [tri | tri] for pair masks

            # ---- input DMAs across three queues, priority order. sync: X'
            # per-k-tile (V projection starts as chunks land) then Wo (needed
            # last). scalar: small consts then Wq/Wk. gpsimd: bv_aug + Wv.
            nc.scalar.dma_start(BQ[:], bqt)
            nc.scalar.dma_start(BK[:], bkt)
            nc.scalar.dma_start(BO[:], bot)
            nc.scalar.dma_start(TRI2[:], tri)
            nc.gpsimd.dma_start(BVB[:], bvb)
            for kt in range(NT):
                nc.sync.dma_start(XT[:, kt * T : (kt + 1) * T], xt[:, kt * T : (kt + 1) * T])
                nc.gpsimd.dma_start(WV[:, kt * VW : (kt + 1) * VW], wv[:, kt * VW : (kt + 1) * VW])
            nc.scalar.dma_start(WQ[:], wq)
            nc.scalar.dma_start(WK[:], wk)
            nc.sync.dma_start(WO[:], wo)

            # ---- PE warmup: dummy matmuls with no DMA dependency so the
            # HAM activity monitor lifts the 1.2GHz cold gate before real
            # work arrives (DUM memset first: it gates the dummies) ----
            DUMW = cpool.tile([128, 128], bf16)
            DUMR = cpool.tile([128, 512], bf16)
            nc.vector.memset(DUMW[:], 1.0)
            nc.vector.memset(DUMR[:], 1.0)

            def dummy(n=512):
                # full-array junk matmul: the HAM activity monitor only lifts
                # the 1.2GHz cold gate for real array occupancy.
                d_ps = pp.tile([128, 512], f32, tag="proj")
                nc.tensor.matmul(
                    d_ps[:, :n], lhsT=DUMW[:], rhs=DUMR[:, :n], start=True, stop=True
                )

            for _ in range(16):
                dummy()

            # ---- V projection: lhsT = X'[kt, jblk] -> V_aug[j, h*65+c].
            # The ones column comes out of the projection itself (Wv col 0,
            # bias 1), so evictions are plain contiguous adds.
            for jt in range(NJ):
                for half in range(2):
                    e0 = half * 390
                    ps = pp.tile([128, 512], f32, tag="proj")
                    for kt in range(NT):
                        nc.tensor.matmul(
                            ps[:, :390],
                            lhsT=XT[:, kt * T + jt * 128 : kt * T + jt * 128 + 128],
                            rhs=WV[:, kt * VW + e0 : kt * VW + e0 + 390],
                            start=(kt == 0),
                            stop=(kt == NT - 1),
                        )
                    nc.vector.tensor_add(
                        VS[:, jt * VW + e0 : jt * VW + e0 + 390],
                        ps[:, :390],
                        BVB[:, e0 : e0 + 390],
                    )

            # ---- Q'/K' projection for one 128-row block nt (2 heads) ----
            def qk_proj(nt):
                for W, Bb, DST in ((WQ, BQ, QS), (WK, BK, KS)):
                    for ic in range(2):
                        ps = pp.tile([128, 512], f32, tag="proj")
                        for kt in range(NT):
                            nc.tensor.matmul(
                                ps[:],
                                lhsT=W[:, nt * E + kt * 128 : nt * E + kt * 128 + 128],
                                rhs=XT[:, kt * T + ic * 512 : kt * T + ic * 512 + 512],
                                start=(kt == 0),
                                stop=(kt == NT - 1),
                            )
                        nc.vector.tensor_scalar_add(
                            DST[:, nt * T + ic * 512 : nt * T + ic * 512 + 512],
                            ps[:],
                            Bb[:, nt : nt + 1],
                        )

            def normalize2(o_e, o_o, nt, ic):
                # softmax denominators live in row 64 (the V_aug ones column).
                # Full-precision reciprocal costs 3.35us on DVE; the ~18-bit
                # approx is plenty, but its BITWISE_NOT seed needs an SBUF
                # operand on hardware, so stage the PSUM rows out first.
                # Both heads' denominators share one reciprocal op.
                dn = rpool.tile([1, 1024], f32, tag="denom")
                nc.vector.tensor_copy(dn[0:1, 0:512], o_e[64:65, :])
                nc.vector.tensor_copy(dn[0:1, 512:1024], o_o[64:65, :])
                r = rpool.tile([1, 1024], f32, tag="recip")
                nc.vector.reciprocal_approx_fast(r[:], dn[:])
                for po, o_ps, src in ((0, o_e, r[0:1, 0:512]), (64, o_o, r[0:1, 512:1024])):
                    rb = rbpool.tile([64, 512], f32, tag="recipb")
                    nc.gpsimd.partition_broadcast(rb[:], src)
                    nc.vector.tensor_mul(
                        OS[po : po + 64, nt * T + ic * 512 : nt * T + ic * 512 + 512],
                        o_ps[0:64, :],
                        rb[:],
                    )

            # ---- attention for one head pair, per 512-col i-chunk.
            # Phase A: paired scores matmuls (even head -> cols 0:512 = bank
            # A, odd head -> 512:1024 = bank B; disjoint 64-row PE tiles run
            # concurrently), one merged exp + tri-mask per j-tile.
            # Phase B: PV accumulation (full array) off the SBUF P tiles. ----
            def pair(nt, ic):
                he, ho = 2 * nt, 2 * nt + 1
                jmax = 4 if ic == 0 else NJ
                p2s = []
                for jt in range(jmax):
                    lo = max(0, jt * 128 - ic * 512)
                    s2 = sp.tile([128, 1024], f32, tag="scores")
                    p2 = ppool.tile([128, 1024], bf16, tag="probs")
                    p2s.append((p2, lo))
                    jb = nt * T + jt * 128
                    qlo = nt * T + ic * 512 + lo
                    qhi = nt * T + (ic + 1) * 512
                    nc.tensor.matmul(
                        s2[:, lo:512],
                        lhsT=KS[0:64, jb : jb + 128],
                        rhs=QS[0:64, qlo:qhi],
                        start=True,
                        stop=True,
                    )
                    nc.tensor.matmul(
                        s2[:, 512 + lo : 1024],
                        lhsT=KS[64:128, jb : jb + 128],
                        rhs=QS[64:128, qlo:qhi],
                        start=True,
                        stop=True,
                    )
                    nc.scalar.activation(
                        p2[:].rearrange("p (h c) -> p h c", h=2)[:, :, lo:512],
                        s2[:].rearrange("p (h c) -> p h c", h=2)[:, :, lo:512],
                        Exp,
                        scale=SCALE,
                    )
                    dl = jt * 128 - ic * 512
                    if dl >= 0:  # diagonal block lives in this i-chunk
                        nc.vector.tensor_mul(
                            p2[:].rearrange("p (h c) -> p h c", h=2)[:, :, dl : dl + 128],
                            p2[:].rearrange("p (h c) -> p h c", h=2)[:, :, dl : dl + 128],
                            TRI2[:].rearrange("p (h c) -> p h c", h=2),
                        )
                o_e = op.tile([128, 512], f32, tag="oaug")
                o_o = op.tile([128, 512], f32, tag="oaug")
                for jt in range(jmax):
                    p2, lo = p2s[jt]
                    nc.tensor.matmul(
                        o_e[0:65, lo:512],
                        lhsT=VS[:, jt * VW + he * 65 : jt * VW + he * 65 + 65],
                        rhs=p2[:, lo:512],
                        start=(jt == 0),
                        stop=(jt == jmax - 1),
                        skip_group_check=True,
                    )
                    nc.tensor.matmul(
                        o_o[0:65, lo:512],
                        lhsT=VS[:, jt * VW + ho * 65 : jt * VW + ho * 65 + 65],
                        rhs=p2[:, 512 + lo : 1024],
                        start=(jt == 0),
                        stop=(jt == jmax - 1),
                        skip_group_check=True,
                    )
                normalize2(o_e, o_o, nt, ic)

            # Interleave: each nt's Q/K projection feeds its head pair; the
            # next nt's projection matmuls keep PE busy while ScalarE runs
            # this pair's exps.
            for nt in range(NT):
                qk_proj(nt)
                pair(nt, 0)
                pair(nt, 1)

            # ---- output projection, transposed: out^T = Wo^T @ O' (+bo).
            # lhsT = Wo[et-block, nt-block] (host-packed), rhs = O' i-chunks.
            # Bias is per-partition -> fused into the eviction tensor_scalar.
            # bf16 out_t, host transposes/upcasts.
            for nt in range(NT):
                fin = fpool.tile([128, T], bf16, tag="fin")
                for ic in range(2):
                    f_ps = pp.tile([128, 512], f32, tag="proj")
                    for et in range(NT):
                        nc.tensor.matmul(
                            f_ps[:],
                            lhsT=WO[:, nt * E + et * 128 : nt * E + et * 128 + 128],
                            rhs=OS[:, et * T + ic * 512 : et * T + ic * 512 + 512],
                            start=(et == 0),
                            stop=(et == NT - 1),
                        )
                    nc.vector.tensor_scalar_add(
                        fin[:, ic * 512 : (ic + 1) * 512], f_ps[:], BO[:, nt : nt + 1]
                    )
                nc.sync.dma_start(out_t[nt * 128 : (nt + 1) * 128, :], fin[:])

    nc.compile()
    return nc


def _get_nc():
    if "nc" not in _CACHE:
        _CACHE["nc"] = _build()
    return _CACHE["nc"]


def _pack_w(w):
    # [768, 768] -> [128, nt*768 + kt*128 + c] = w[kt*128+p, nt*128+c]
    return np.ascontiguousarray(
        w.reshape(NT, 128, NT, 128).transpose(1, 2, 0, 3).reshape(128, NT * E)
    )


def _make_in_maps(inputs):
    bf = ml_dtypes.bfloat16
    x = np.asarray(inputs["x"], np.float32)
    wv4 = np.asarray(inputs["Wv"], np.float32).reshape(E, H, DH)
    wv_aug = np.zeros((E, H, 65), np.float32)
    wv_aug[:, :, :DH] = wv4
    bv_aug = np.zeros((H, 65), np.float32)
    bv_aug[:, :DH] = np.asarray(inputs["bv"], np.float32).reshape(H, DH)
    bv_aug[:, DH] = 1.0
    shared = {
        "wq": _pack_w(np.asarray(inputs["Wq"], np.float32)).astype(bf),
        "wk": _pack_w(np.asarray(inputs["Wk"], np.float32)).astype(bf),
        "wo": _pack_w(np.asarray(inputs["Wo"], np.float32)).astype(bf),
        "wv": np.ascontiguousarray(
            wv_aug.reshape(NT, 128, VW).transpose(1, 0, 2).reshape(128, NT * VW)
        ).astype(bf),
        "bvb": np.ascontiguousarray(
            np.broadcast_to(bv_aug.reshape(1, VW), (128, VW))
        ).astype(bf),
        "bqt": np.ascontiguousarray(
            np.asarray(inputs["bq"], np.float32).reshape(NT, 128).T
        ),
        "bkt": np.ascontiguousarray(
            np.asarray(inputs["bk"], np.float32).reshape(NT, 128).T
        ),
        "bot": np.ascontiguousarray(
            np.asarray(inputs["bo"], np.float32).reshape(NT, 128).T
        ),
        "tri": np.tile(np.triu(np.ones((128, 128), np.float32)), (1, 2)).astype(bf),
    }
    xs = []
    for b in range(B):
        xp = np.ascontiguousarray(
            x[b].T.reshape(NT, 128, T).transpose(1, 0, 2).reshape(128, NT * T)
        ).astype(bf)
        xs.append(dict(shared, xt=xp))
    return xs


def _run(inputs, trace=False):
    from concourse import bass_utils

    nc = _get_nc()
    res = bass_utils.run_bass_kernel_spmd(
        nc, _make_in_maps(inputs), core_ids=list(range(B)), trace=trace
    )
    out = np.stack(
        [np.asarray(res.results[c]["out_t"]).astype(np.float32).T for c in range(B)]
    )
    return out, res


def kernel(**inputs) -> np.ndarray:
    out, _ = _run(inputs, trace=False)
    return out
